# revision 14
# baseline (speedup 1.0000x reference)
"""Multi-head attention (B=2, S=2048, D=1024, H=16) on 8 Trainium2 NeuronCores.

Sharding: batch x head-group. Core c handles batch b = c//4 and heads
[4*(c%4), 4*(c%4)+4) (a 256-wide slice of the QKV projection output and the
matching 256-row slice of Wo). Each core computes its partial output
projection; a 4-way ReduceScatter per batch group sums the partials and
leaves each core with row blocks of the final output, which the host
reassembles.

v2 changes vs v1:
  - Inputs arrive host-transposed ([D, S] fp16) -> plain chunked DMAs at
    full HBM bandwidth instead of serialized DMA-transposes; chunks flow
    through small rotating pools so compute starts ~3us in.
  - Scores matmuls are row-tiled: the two heads of a pair occupy PE array
    row groups 0-63 / 64-127 (64-deep contraction each) and run
    concurrently -- no zero-padded K, half the PE slots.
  - K projection is interleaved with the first score waves; Q projection
    and the previous chunk's output projection are tucked inside later
    score-exp windows so ScalarE (the exp stream, the critical engine)
    never starves.
  - PSUM->SBUF evacuations that would land in the exp window run on the
    Vector engine; ScalarE keeps only the exp stream.
  - reciprocal_approx_fast instead of the 8-cycle/elem iterative divide.
  - The last q-chunk's ReduceScatter is split in two to shorten the
    end-of-kernel collective tail.
"""

import numpy as np

import concourse.bass as bass  # noqa: F401  (engine namespaces via nc)
import concourse.mybir as mybir
import concourse.tile as tile
from concourse import bacc
from concourse.bass import _add_dep_helper
from concourse.bass_utils import run_bass_kernel_spmd

F32 = mybir.dt.float32
F16 = mybir.dt.float16
AF = mybir.ActivationFunctionType

B, S, D = 2, 2048, 1024
H, DH = 16, 64
NCORES = 8
GPB = 4                # cores per batch group
HPC = H // GPB         # heads per core
DS = HPC * DH          # 256: per-core slice of the projection output
P = 128
NDT = D // P           # 8 d_model tiles
NTT = S // P           # 16 token tiles
QCH = 512              # q-chunk (PSUM bank = 512 fp32)
NQC = S // QCH         # 4
NKT = S // P           # 16 k tiles
NW = NKT // 2          # 8 score waves per (qc, pr), 2 k-tiles each
SCALE = float(1.0 / np.sqrt(DH))

REPLICA_GROUPS = [[0, 1, 2, 3], [4, 5, 6, 7]]

_CACHED_NC = None


def _build_module():
    nc = bacc.Bacc("TRN2", target_bir_lowering=False, debug=False,
                   num_devices=NCORES)

    # host pre-shuffled activations: [chunk, p, dt, t] fp16 so each chunk
    # DMA is one 8KB-per-partition contiguous line (full HBM bandwidth)
    xq_d = nc.dram_tensor("xq", [NQC, P, NDT, QCH], F16, kind="ExternalInput")
    xk_d = nc.dram_tensor("xk", [NQC, P, NDT, QCH], F16, kind="ExternalInput")
    xv_d = nc.dram_tensor("xv", [NQC, P, NDT, QCH], F16, kind="ExternalInput")
    wq_d = nc.dram_tensor("wq", [D, DS], F16, kind="ExternalInput")
    wk_d = nc.dram_tensor("wk", [D, DS], F16, kind="ExternalInput")
    wv_d = nc.dram_tensor("wv", [D, DS], F16, kind="ExternalInput")
    wo_d = nc.dram_tensor("wo", [DS, D], F16, kind="ExternalInput")
    bq_d = nc.dram_tensor("bq", [DS, 1], F32, kind="ExternalInput")
    bk_d = nc.dram_tensor("bk", [DS, 1], F32, kind="ExternalInput")
    bv_d = nc.dram_tensor("bv", [1, DS], F32, kind="ExternalInput")
    bo_d = nc.dram_tensor("bo", [1, D], F32, kind="ExternalInput")

    # out rows: qc j -> rows [j*128, (j+1)*128)
    out_d = nc.dram_tensor("out", [S // GPB, D], F16, kind="ExternalOutput")
    partial_cs = [nc.dram_tensor(f"partial{j}", [4 * P, D], F16)
                  for j in range(4)]
    rs_cs = [nc.dram_tensor(f"rs_out{j}", [P, D], F16) for j in range(4)]

    with tile.TileContext(nc) as tc:
        with (
            tc.tile_pool(name="cst", bufs=1) as cst,
            tc.tile_pool(name="xin", bufs=2) as xinp,
            tc.tile_pool(name="exp", bufs=38) as expp,
            tc.tile_pool(name="rcp", bufs=2) as rcpp,
            tc.tile_pool(name="osb", bufs=3) as osbp,
            tc.tile_pool(name="psB", bufs=3, space="PSUM") as psB,
            tc.tile_pool(name="psC", bufs=1, space="PSUM") as psC,
        ):
            # Total PE ordering: chain every matmul to its predecessor
            # (nosync = scheduling-order only); on a PE-array tiling-mode
            # change (plain / row-tiled / col-tiled) add a semaphore edge so
            # the array drains before the mode flips.
            _real_matmul = nc.tensor.matmul
            _prev_mm = {"inst": None, "mode": None}

            def mm(mode, out, lhsT, rhs, **kw):
                inst = _real_matmul(out, lhsT, rhs, **kw)
                if _prev_mm["inst"] is not None:
                    _add_dep_helper(
                        inst.ins, _prev_mm["inst"].ins,
                        sync=(mode != _prev_mm["mode"]),
                        reason="pe-mode-order")
                _prev_mm["inst"] = inst
                _prev_mm["mode"] = mode
                return inst

            # ---- weights / biases (gpsimd DMA queue) ----
            wq_t = cst.tile([P, NDT, DS], F16, tag="wq")
            wk_t = cst.tile([P, NDT, DS], F16, tag="wk")
            wv_t = cst.tile([P, NDT, DS], F16, tag="wv")
            wo_t = cst.tile([P, 2, D], F16, tag="wo")
            nc.gpsimd.dma_start(wk_t[:], wk_d.rearrange("(a p) n -> p a n", p=P))
            nc.gpsimd.dma_start(wq_t[:], wq_d.rearrange("(a p) n -> p a n", p=P))
            nc.gpsimd.dma_start(wv_t[:], wv_d.rearrange("(a p) n -> p a n", p=P))
            nc.gpsimd.dma_start(wo_t[:], wo_d.rearrange("(a p) n -> p a n", p=P))

            bq_t = cst.tile([P, 2, 1], F32, tag="bq")
            bk_t = cst.tile([P, 2, 1], F32, tag="bk")
            nc.gpsimd.dma_start(bq_t[:], bq_d.rearrange("(a p) o -> p a o", p=P))
            nc.gpsimd.dma_start(bk_t[:], bk_d.rearrange("(a p) o -> p a o", p=P))

            bv_row = cst.tile([1, DS], F32, tag="bvr")
            bo_row = cst.tile([1, D], F32, tag="bor")
            nc.gpsimd.dma_start(bv_row[:], bv_d[:])
            nc.gpsimd.dma_start(bo_row[:], bo_d[:])
            bv_b = cst.tile([P, DS], F32, tag="bvb")
            bo_b = cst.tile([P, D], F32, tag="bob")
            nc.gpsimd.partition_broadcast(bv_b[:], bv_row[:])
            nc.gpsimd.partition_broadcast(bo_b[:], bo_row[:])

            ones_t = cst.tile([P, DH], F16, tag="ones")
            nc.vector.memset(ones_t[:], 1.0)
            # preload the exp table set before the attention stream needs it
            warm_t = cst.tile([P, DH], F16, tag="warm")
            nc.scalar.activation(warm_t[:], ones_t[:], AF.Exp, scale=SCALE)

            def load_chunk(tag, queue, x_d, c):
                t = xinp.tile([P, NDT, QCH], F16, tag=tag, name=f"x{tag}{c}")
                queue.dma_start(t[:], x_d[c])
                return t

            # ---- resident activations ----
            qt_t = cst.tile([P, 2, S], F16, tag="qt")   # Q^T  (pair, t)
            kt_t = cst.tile([P, 2, S], F16, tag="kt")   # K^T  (pair, t)
            v_t = cst.tile([P, NTT, DS], F16, tag="vt")  # V token-major
            an_t = cst.tile([P, 2, S], F16, tag="an")   # attn_norm^T

            # ---- feature-major projection: out^T[ds, t] (Q^T / K^T) ----
            def proj_T(dst, w_t, b_t, xt_c, tci, on_act):
                ts0 = tci * QCH
                ps = psB.tile([P, 2 * QCH], F32, tag="sc")
                for dot in range(2):
                    col = slice(dot * QCH, (dot + 1) * QCH)
                    for dt in range(NDT):
                        mm(
                            "plain", ps[:, col],
                            w_t[:, dt, dot * P:(dot + 1) * P],
                            xt_c[:, dt, :],
                            start=(dt == 0), stop=(dt == NDT - 1),
                        )
                for dot in range(2):
                    col = slice(dot * QCH, (dot + 1) * QCH)
                    if on_act:
                        nc.scalar.activation(
                            dst[:, dot, ts0:ts0 + QCH], ps[:, col],
                            AF.Identity, bias=b_t[:, dot, :])
                    else:
                        nc.vector.tensor_scalar_add(
                            dst[:, dot, ts0:ts0 + QCH], ps[:, col],
                            b_t[:, dot, :])

            def v_proj(xt_c, tt):
                ps = psC.tile([P, DS], F32,
                              tag=("acc" if tt % 2 == 0 else "sum"))
                lo = (tt % 4) * P
                for dt in range(NDT):
                    mm(
                        "plain", ps[:],
                        xt_c[:, dt, lo:lo + P],
                        wv_t[:, dt, :],
                        start=(dt == 0), stop=(dt == NDT - 1),
                    )
                nc.vector.tensor_add(v_t[:, tt, :], ps[:], bv_b[:, :])

            # ---- scores wave: k-tiles {2w, 2w+1} for both heads of pr ----
            # Row-tiled: head-even contracts over partitions 0-63 (array row
            # group 0-1), head-odd over 64-127 -- the two matmuls of a k-tile
            # run concurrently in the array.
            def score_wave(qc, pr, w, etiles):
                qs = qc * QCH
                t0 = psB.tile([P, 2 * QCH], F32, tag="sc")
                t1 = psB.tile([P, 2 * QCH], F32, tag="sc")
                for j in range(2):
                    ks = (2 * w + j) * P
                    col = slice(j * QCH, (j + 1) * QCH)
                    mm(
                        "row", t0[:, col],
                        kt_t[0:64, pr, ks:ks + P],
                        qt_t[0:64, pr, qs:qs + QCH],
                        start=True, stop=True)
                    mm(
                        "row", t1[:, col],
                        kt_t[64:128, pr, ks:ks + P],
                        qt_t[64:128, pr, qs:qs + QCH],
                        start=True, stop=True)
                e0 = expp.tile([P, 2 * QCH], F16, tag="exp", name=f"e0_{qc}_{pr}_{w}")
                e1 = expp.tile([P, 2 * QCH], F16, tag="exp", name=f"e1_{qc}_{pr}_{w}")
                nc.scalar.activation(e0[:], t0[:], AF.Exp, scale=SCALE)
                nc.scalar.activation(e1[:], t1[:], AF.Exp, scale=SCALE)
                etiles.append((e0, e1))

            # ---- attn@V + sums (col-tiled 128x64), then normalize ----
            # Emitted in kt-range chunks so the work can interleave with
            # score waves; within a chunk the sm chain runs before the acc
            # chain (constant ones lhsT -> cheap weight loads back-to-back).
            def phase2_chunk(st8, kt0, kt1, pr, etiles):
                acc, sm = st8
                h0 = 2 * pr
                h1 = 2 * pr + 1
                for kt in range(kt0, kt1):
                    e0, e1 = etiles[kt // 2]
                    col = slice((kt % 2) * QCH, (kt % 2 + 1) * QCH)
                    st = (kt == 0)
                    sp = (kt == NKT - 1)
                    mm(
                        "col", sm[0:64, :], ones_t[:], e0[:, col],
                        start=st, stop=sp,
                        tile_position=(0, 0), skip_group_check=True)
                    mm(
                        "col", sm[64:128, :], ones_t[:], e1[:, col],
                        start=st, stop=sp,
                        tile_position=(0, 64), skip_group_check=True)
                for kt in range(kt0, kt1):
                    e0, e1 = etiles[kt // 2]
                    col = slice((kt % 2) * QCH, (kt % 2 + 1) * QCH)
                    st = (kt == 0)
                    sp = (kt == NKT - 1)
                    mm(
                        "col", acc[0:64, :], v_t[:, kt, h0 * DH:(h0 + 1) * DH],
                        e0[:, col], start=st, stop=sp,
                        tile_position=(0, 0), skip_group_check=True)
                    mm(
                        "col", acc[64:128, :], v_t[:, kt, h1 * DH:(h1 + 1) * DH],
                        e1[:, col], start=st, stop=sp,
                        tile_position=(0, 64), skip_group_check=True)

            def phase2_start():
                acc = psC.tile([P, QCH], F32, tag="acc")
                sm = psC.tile([P, QCH], F32, tag="sum")
                return (acc, sm)

            def phase2_fin(st8, qc, pr):
                acc, sm = st8
                qs = qc * QCH
                rc = rcpp.tile([P, QCH], F32, tag="rcp")
                nc.vector.reciprocal_approx_fast(rc[:], sm[:])
                nc.vector.tensor_mul(an_t[:, pr, qs:qs + QCH], acc[:], rc[:])

            # ---- output projection for one token tile ----
            def o_proj(tt, dst_d, dst_row):
                po = psB.tile([P, 2 * QCH], F32, tag="sc")
                for half in range(2):
                    for pr in range(2):
                        mm(
                            "plain", po[:, half * QCH:(half + 1) * QCH],
                            an_t[:, pr, tt * P:(tt + 1) * P],
                            wo_t[:, pr, half * QCH:(half + 1) * QCH],
                            start=(pr == 0), stop=(pr == 1))
                ob = osbp.tile([P, D], F16, tag="osb")
                nc.vector.tensor_add(ob[:], po[:], bo_b[:])
                nc.sync.dma_start(
                    dst_d[dst_row * P:(dst_row + 1) * P, :], ob[:])

            def rs_chunk(qc):
                nc.gpsimd.collective_compute(
                    "ReduceScatter", mybir.AluOpType.add,
                    replica_groups=REPLICA_GROUPS,
                    ins=[partial_cs[qc][:]], outs=[rs_cs[qc][:]])
                # gpsimd-queue DMA: must not block the sync queue, where
                # the next chunk's partial DMAs live (else RSs serialize)
                nc.gpsimd.dma_start(out_d[qc * P:(qc + 1) * P, :],
                                    rs_cs[qc][:])

            # ================= emission =================
            # v3-style cascade: batched score blocks with the next chunk's
            # pr0 scores emitted before the current chunk's pr1 phase2, so
            # ScalarE always has a pending score stream. (A fully
            # interleaved schedule was tried and ran slower: overlapping
            # every engine inflated exp latency ~20%.)
            xk_cs = [load_chunk("xk", nc.sync, xk_d, 0)]
            xq_cs = [load_chunk("xq", nc.sync, xq_d, 0)]

            proj_T(kt_t, wk_t, bk_t, xk_cs[0], 0, on_act=True)
            proj_T(qt_t, wq_t, bq_t, xq_cs[0], 0, on_act=True)

            ets = {}

            # (0,0): remaining K projection interleaved
            ets[(0, 0)] = []
            for w in range(2):
                score_wave(0, 0, w, ets[(0, 0)])
            for tc in range(1, 4):
                xk_cs.append(load_chunk("xk", nc.sync, xk_d, tc))
                proj_T(kt_t, wk_t, bk_t, xk_cs[tc], tc, on_act=False)
                for w in range(2 * tc, 2 * tc + 2):
                    score_wave(0, 0, w, ets[(0, 0)])

            # (0,1): V projection interleaved (2 tts per wave)
            ets[(0, 1)] = []
            xv_cs = []
            for w in range(NW):
                score_wave(0, 1, w, ets[(0, 1)])
                for tt in (2 * w, 2 * w + 1):
                    if tt % 4 == 0:
                        xv_cs.append(load_chunk("xv", nc.gpsimd, xv_d, tt // 4))
                    v_proj(xv_cs[tt // 4], tt)

            def p2_full(qc, pr):
                p2s = phase2_start()
                phase2_chunk(p2s, 0, NKT, pr, ets[(qc, pr)])
                phase2_fin(p2s, qc, pr)

            p2_full(0, 0)
            xq_cs.append(load_chunk("xq", nc.sync, xq_d, 1))
            proj_T(qt_t, wq_t, bq_t, xq_cs[1], 1, on_act=False)
            ets[(1, 0)] = []
            for w in range(NW):
                score_wave(1, 0, w, ets[(1, 0)])
            p2_full(0, 1)

            for qc in range(1, NQC):
                ets[(qc, 1)] = []
                if qc < 3:
                    for w in range(NW):
                        score_wave(qc, 1, w, ets[(qc, 1)])
                    p2_full(qc, 0)
                    for tt4 in range(4):
                        o_proj((qc - 1) * 4 + tt4, partial_cs[qc - 1], tt4)
                    rs_chunk(qc - 1)
                    xq_cs.append(load_chunk("xq", nc.sync, xq_d, qc + 1))
                    proj_T(qt_t, wq_t, bq_t, xq_cs[qc + 1], qc + 1,
                           on_act=False)
                    ets[(qc + 1, 0)] = []
                    for w in range(NW):
                        score_wave(qc + 1, 0, w, ets[(qc + 1, 0)])
                    p2_full(qc, 1)
                else:
                    # last chunk: interleave phase2(3,0) + boundary work into
                    # the (3,1) score waves so the tail after the last wave is
                    # just phase2(3,1) + O-proj + the final ReduceScatter
                    p2s30 = phase2_start()
                    for w in range(NW):
                        score_wave(3, 1, w, ets[(3, 1)])
                        if w == 1:
                            phase2_chunk(p2s30, 0, NKT // 2, 0, ets[(3, 0)])
                        elif w == 3:
                            phase2_chunk(p2s30, NKT // 2, NKT, 0,
                                         ets[(3, 0)])
                        elif w == 5:
                            phase2_fin(p2s30, 3, 0)
                        elif w == 7:
                            for tt4 in range(4):
                                o_proj(8 + tt4, partial_cs[2], tt4)
                            rs_chunk(2)
                    p2_full(3, 1)

            for tt4 in range(4):
                o_proj(12 + tt4, partial_cs[3], tt4)
            rs_chunk(3)

    nc.compile()
    return nc


def _get_nc():
    global _CACHED_NC
    if _CACHED_NC is None:
        _CACHED_NC = _build_module()
    return _CACHED_NC


def _make_in_maps(query, key, value, Wq, bq, Wk, bk, Wv, bv, Wo, bo):
    query = np.asarray(query, dtype=np.float32)
    key = np.asarray(key, dtype=np.float32)
    value = np.asarray(value, dtype=np.float32)
    Wq = np.asarray(Wq, dtype=np.float32)
    Wk = np.asarray(Wk, dtype=np.float32)
    Wv = np.asarray(Wv, dtype=np.float32)
    Wo = np.asarray(Wo, dtype=np.float32)
    bq = np.asarray(bq, dtype=np.float32)
    bk = np.asarray(bk, dtype=np.float32)
    bv = np.asarray(bv, dtype=np.float32)
    bo = np.asarray(bo, dtype=np.float32)

    in_maps = []
    for c in range(NCORES):
        b = c // GPB
        g = c % GPB
        sl = slice(g * DS, (g + 1) * DS)
        def shuf(x):
            # [S, D] -> [chunk, p, dt, t]: per-chunk 8KB contiguous lines
            xT = np.ascontiguousarray(x.T).astype(np.float16)
            return np.ascontiguousarray(
                xT.reshape(NDT, P, NQC, QCH).transpose(2, 1, 0, 3))

        in_maps.append({
            "xq": shuf(query[b]),
            "xk": shuf(key[b]),
            "xv": shuf(value[b]),
            "wq": Wq[:, sl].astype(np.float16),
            "wk": Wk[:, sl].astype(np.float16),
            "wv": Wv[:, sl].astype(np.float16),
            "wo": Wo[sl, :].astype(np.float16),
            "bq": bq[sl].reshape(DS, 1).copy(),
            "bk": bk[sl].reshape(DS, 1).copy(),
            "bv": bv[sl].reshape(1, DS).copy(),
            "bo": (bo if g == 0 else np.zeros_like(bo)).reshape(1, D).copy(),
        })
    return in_maps


def run(inputs, trace=False, trace_cores=None):
    """Run the SPMD kernel; returns (full_output, BassKernelResults)."""
    nc = _get_nc()
    in_maps = _make_in_maps(**inputs)
    res = run_bass_kernel_spmd(
        nc, in_maps, core_ids=list(range(NCORES)), trace=trace,
        trace_cores=trace_cores)
    out = np.empty((B, S, D), dtype=np.float32)
    for c in range(NCORES):
        b = c // GPB
        g = c % GPB
        o = res.results[c]["out"].astype(np.float32)
        for j in range(4):
            out[b, j * 512 + g * P:j * 512 + (g + 1) * P, :] = \
                o[j * P:(j + 1) * P, :]
    return out, res


def kernel(**inputs):
    out, _ = run(inputs, trace=False)
    return out
